# Initial kernel scaffold
#
"""FCCapsuleLayer (dynamic routing, 3 iters) Trainium2 Bass kernel.

Sharding: data-parallel over batch, 8 cores x 4 batches. Per core 1024
positions (4*16*16). Routing is local per position, so no cross-core
communication is needed.

Per-core program (8 blocks of 128 positions, pos on SBUF partitions):
  - votes[pos, i, nc, cd] = x[pos,i,:] @ W    via PE: for each i, one
    matmul with stationary xT_i [a=16, pos=128] and moving W [16, 160].
    PSUM->SBUF evacuation on ScalarE (keeps DVE free).
  - 3 routing iterations on DVE with strided-view segmented reduces:
      iter1: route uniform -> preact = 0.1*sum_i votes + b
      agreement: tmp = votes * act  -> reduce over cd -> logits
      softmax over nc; preact = reduce_i (votes * route) + b; squash.
Host side: shard, pre-transpose x to [a, blk, i, pos]; gather outputs.
"""

from contextlib import ExitStack

import numpy as np

import concourse.bacc as bacc
import concourse.bass as bass
import concourse.tile as tile
from concourse import bass_utils, mybir

F32 = mybir.dt.float32
AX = mybir.AxisListType
OP = mybir.AluOpType

B, H, Wd, IC, IA = 32, 16, 16, 32, 16
NC, CD = 10, 16
NCD = NC * CD  # 160
NCORES = 8
BPC = B // NCORES          # batches per core
POS = BPC * H * Wd         # 1024 positions per core
PB = 128                   # positions per block
NBLK = POS // PB           # 8
EPS = 1e-7
IGRP = 3                   # i's per PSUM tile (3*160*4B = 1920B < 2KB bank)

_PROG_CACHE = {}


def _squash_emit(nc, pool, pre, tag, eps_s=None):
    """pre: [128, NCD] tile AP viewed [p, nc, cd]. Returns act tile [128, NCD]."""
    psq = pool.tile([PB, NCD], F32, tag=f"psq{tag}")
    nc.scalar.square(psq[:], pre[:])
    sq = pool.tile([PB, NC], F32, tag=f"sq{tag}")
    nc.vector.tensor_reduce(
        sq[:], psq[:].rearrange("p (n c) -> p n c", n=NC, c=CD),
        axis=AX.X, op=OP.add)
    t1 = pool.tile([PB, NC], F32, tag=f"t1{tag}")
    nc.vector.tensor_scalar_add(t1[:], sq[:], 1.0)
    r1 = pool.tile([PB, NC], F32, tag=f"r1{tag}")
    nc.vector.reciprocal(r1[:], t1[:])
    fa = pool.tile([PB, NC], F32, tag=f"fa{tag}")
    nc.vector.tensor_mul(fa[:], sq[:], r1[:])          # sq/(1+sq)
    lg = pool.tile([PB, NC], F32, tag=f"lg{tag}")
    nc.scalar.activation(lg[:], sq[:], mybir.ActivationFunctionType.Ln,
                         bias=eps_s[:])                # ln(sq+eps)
    r2 = pool.tile([PB, NC], F32, tag=f"r2{tag}")
    nc.scalar.activation(r2[:], lg[:], mybir.ActivationFunctionType.Exp,
                         scale=-0.5)                   # rsqrt(sq+eps)
    f = pool.tile([PB, NC], F32, tag=f"f{tag}")
    nc.vector.tensor_mul(f[:], fa[:], r2[:])
    act = pool.tile([PB, NCD], F32, tag=f"act{tag}")
    fb = f[:].unsqueeze(2).broadcast_to((PB, NC, CD))
    nc.vector.tensor_mul(
        act[:].rearrange("p (n c) -> p n c", n=NC, c=CD),
        pre[:].rearrange("p (n c) -> p n c", n=NC, c=CD), fb)
    return act


def _softmax_emit(nc, pool, logits, tag):
    """logits: [128, IC*NC] viewed [p, i, nc]. Returns route tile [128, IC*NC]."""
    e = pool.tile([PB, IC * NC], F32, tag=f"e{tag}")
    ev = e[:].rearrange("p (i n) -> p i n", i=IC, n=NC)
    nc.scalar.activation(e[:], logits[:], mybir.ActivationFunctionType.Exp)
    d = pool.tile([PB, IC], F32, tag=f"d{tag}")
    nc.vector.tensor_reduce(d[:], ev, axis=AX.X, op=OP.add)
    r = pool.tile([PB, IC], F32, tag=f"r{tag}")
    nc.vector.reciprocal(r[:], d[:])
    route = pool.tile([PB, IC * NC], F32, tag=f"route{tag}")
    rb = r[:].unsqueeze(2).broadcast_to((PB, IC, NC))
    nc.vector.tensor_mul(
        route[:].rearrange("p (i n) -> p i n", i=IC, n=NC), ev, rb)
    return route


def _build_program():
    nc = bacc.Bacc("TRN2", target_bir_lowering=False, debug=False,
                   enable_asserts=False, num_devices=NCORES)
    xT_d = nc.dram_tensor("xT", [IA, NBLK * IC * PB], F32,
                          kind="ExternalInput").ap()
    w_d = nc.dram_tensor("w", [IA, NCD], F32, kind="ExternalInput").ap()
    bb_d = nc.dram_tensor("bb", [PB, NCD], F32, kind="ExternalInput").ap()
    out_d = nc.dram_tensor("out", [POS, NCD], F32, kind="ExternalOutput").ap()

    with tile.TileContext(nc) as tc, ExitStack() as ctx:
        const = ctx.enter_context(tc.tile_pool(name="const", bufs=1))
        w_s = const.tile([IA, NCD], F32)
        nc.sync.dma_start(w_s[:], w_d)
        bb_s = const.tile([PB, NCD], F32)
        nc.sync.dma_start(bb_s[:], bb_d)
        eps_s = const.tile([PB, 1], F32)
        nc.vector.memset(eps_s[:], EPS)
        zero_s = const.tile([PB, 1], F32)
        nc.vector.memset(zero_s[:], 0.0)
        nc.const_aps.aps[(F32, 0.0)] = zero_s[:]

        xt_pool = ctx.enter_context(tc.tile_pool(name="xt", bufs=2))
        votes_pool = ctx.enter_context(tc.tile_pool(name="votes", bufs=2))
        tmp_pool = ctx.enter_context(tc.tile_pool(name="tmp", bufs=3))
        sm = ctx.enter_context(tc.tile_pool(name="small", bufs=4))
        psum = ctx.enter_context(tc.tile_pool(name="ps", bufs=6, space="PSUM"))
        spsum = ctx.enter_context(tc.tile_pool(name="sps", bufs=2, space="PSUM"))

        def emit_front(blk):
            """Votes (PE) + evacuation (ScalarE) + iter-1 squash (small DVE)
            + B1 agreement multiply (GPSIMD). Emitted one block ahead so
            these fill the other engines while DVE grinds block blk-1."""
            xt = xt_pool.tile([IA, IC * PB], F32)
            nc.sync.dma_start(xt[:], xT_d[:, blk * IC * PB:(blk + 1) * IC * PB])
            votes = votes_pool.tile([PB, IC * NCD], F32)
            sps = spsum.tile([PB, NCD], F32, tag="sps")
            i = 0
            while i < IC:
                ni = min(IGRP, IC - i)
                ps = psum.tile([PB, IGRP * NCD], F32, tag="vps")
                for k in range(ni):
                    nc.tensor.matmul(
                        ps[:, k * NCD:(k + 1) * NCD],
                        lhsT=xt[:, (i + k) * PB:(i + k + 1) * PB],
                        rhs=w_s[:], start=True, stop=True)
                    nc.tensor.matmul(
                        sps[:], lhsT=xt[:, (i + k) * PB:(i + k + 1) * PB],
                        rhs=w_s[:], start=(i + k == 0), stop=(i + k == IC - 1),
                        skip_group_check=True)
                nc.scalar.copy(votes[:, i * NCD:(i + ni) * NCD],
                               ps[:, :ni * NCD])
                i += ni

            v_inc = votes[:].rearrange("p (i n c) -> p i n c", i=IC, n=NC, c=CD)

            # iter 1: uniform route
            pre = sm.tile([PB, NCD], F32, tag="pre")
            nc.vector.scalar_tensor_tensor(
                pre[:], sps[:], 1.0 / NC, bb_s[:], op0=OP.mult, op1=OP.add)
            act = _squash_emit(nc, sm, pre, "a", eps_s)

            # agreement 1 multiply. Steady state: GPSIMD + cd-fold 16->4,
            # fully off the DVE chain (prefetched one block ahead). Block 0
            # has no prior block to overlap with, so the slow GPSIMD path
            # would sit on the critical prologue: do it on DVE instead.
            tmp = tmp_pool.tile([PB, IC * NCD], F32, tag="tmp")
            ab = act[:].rearrange("p (n c) -> p n c", n=NC, c=CD).unsqueeze(1) \
                .broadcast_to((PB, IC, NC, CD))
            eng = nc.vector if blk == 0 else nc.gpsimd
            eng.tensor_mul(
                tmp[:].rearrange("p (i n c) -> p i n c", i=IC, n=NC, c=CD),
                v_inc, ab)
            if blk > 0:
                t4 = tmp[:].rearrange("p (i n f c) -> p i n f c", i=IC, n=NC,
                                      f=2, c=8)
                nc.gpsimd.tensor_add(t4[:, :, :, 0, :], t4[:, :, :, 0, :],
                                     t4[:, :, :, 1, :])
                t8 = tmp[:].rearrange("p (i n f c) -> p i n f c", i=IC, n=NC,
                                      f=4, c=4)
                nc.gpsimd.tensor_add(t8[:, :, :, 0, :], t8[:, :, :, 0, :],
                                     t8[:, :, :, 1, :])
            return votes, v_inc, tmp, blk

        def emit_back(blk, votes, v_inc, tmp, fb):
            logits = sm.tile([PB, IC * NC], F32, tag="logits")
            tq = tmp[:].rearrange("p (i n c) -> p i n c", i=IC, n=NC, c=CD)
            nfold = CD if fb == 0 else 4
            nc.vector.tensor_reduce(
                logits[:].rearrange("p (i n) -> p i n", i=IC, n=NC),
                tq[:, :, :, 0:nfold], axis=AX.X, op=OP.add)

            for it in (2, 3):
                route = _softmax_emit(nc, sm, logits, f"it{it}")
                rb = route[:].rearrange("p (i n) -> p i n", i=IC, n=NC).unsqueeze(3) \
                    .broadcast_to((PB, IC, NC, CD))
                tmp2 = tmp_pool.tile([PB, IC * NCD], F32, tag="tmp")
                nc.vector.tensor_mul(
                    tmp2[:].rearrange("p (i n c) -> p i n c",
                                      i=IC, n=NC, c=CD),
                    v_inc, rb)
                pre2 = sm.tile([PB, NCD], F32, tag="pre")
                nc.vector.tensor_reduce(
                    pre2[:].rearrange("p (n c) -> p n c", n=NC, c=CD),
                    tmp2[:].rearrange("p (i n c) -> p n c i",
                                      i=IC, n=NC, c=CD),
                    axis=AX.X, op=OP.add)
                nc.vector.tensor_add(pre2[:], pre2[:], bb_s[:])
                act = _squash_emit(nc, sm, pre2, "a", eps_s)
                if it < 3:
                    tmp3 = tmp_pool.tile([PB, IC * NCD], F32, tag="tmp")
                    ab2 = act[:].rearrange("p (n c) -> p n c", n=NC, c=CD).unsqueeze(1) \
                        .broadcast_to((PB, IC, NC, CD))
                    nc.vector.tensor_mul(
                        tmp3[:].rearrange("p (i n c) -> p i n c",
                                          i=IC, n=NC, c=CD),
                        v_inc, ab2)
                    agree = sm.tile([PB, IC * NC], F32, tag="agree")
                    nc.vector.tensor_reduce(
                        agree[:].rearrange("p (i n) -> p i n", i=IC, n=NC),
                        tmp3[:].rearrange("p (i n c) -> p i n c",
                                          i=IC, n=NC, c=CD),
                        axis=AX.X, op=OP.add)
                    logits2 = sm.tile([PB, IC * NC], F32, tag="logits")
                    nc.vector.tensor_add(logits2[:], logits[:], agree[:])
                    logits = logits2

            nc.sync.dma_start(out_d[blk * PB:(blk + 1) * PB, :], act[:])

        state = {}
        for blk in range(NBLK + 1):
            if blk < NBLK:
                state[blk] = emit_front(blk)
            if blk >= 1:
                v, vi, t, fb = state.pop(blk - 1)
                emit_back(blk - 1, v, vi, t, fb)
    # Pin every ScalarE activation to the one table set that contains all
    # functions we use (exp, ln, square, copy, identity) so the act-table
    # insertion pass emits a single hoisted load instead of thrashing
    # between sets on every softmax/squash (~2.7us per reload).
    _orig_gat = bacc.get_activation_tables
    _ONE_SET = "natural_log_exp_and_others"

    def _pinned(arch):
        tabs = _orig_gat(arch)
        return {k: (v if k == _ONE_SET else set()) for k, v in tabs.items()}

    bacc.get_activation_tables = _pinned
    try:
        nc.compile()
    finally:
        bacc.get_activation_tables = _orig_gat
    return nc


def _get_program():
    if "nc" not in _PROG_CACHE:
        _PROG_CACHE["nc"] = _build_program()
    return _PROG_CACHE["nc"]


def kernel(input_tensor: np.ndarray, W: np.ndarray, b: np.ndarray,
           **_ignored) -> np.ndarray:
    nc = _get_program()
    x = np.asarray(input_tensor, np.float32)
    Wf = np.ascontiguousarray(np.asarray(W, np.float32))
    bb = np.ascontiguousarray(
        np.broadcast_to(np.asarray(b, np.float32).reshape(1, NCD), (PB, NCD)))

    in_maps = []
    for c in range(NCORES):
        xc = x[c * BPC:(c + 1) * BPC].reshape(POS, IC, IA)
        # [pos, i, a] -> [a, blk, i, pos]
        xT = xc.reshape(NBLK, PB, IC, IA).transpose(3, 0, 2, 1)
        in_maps.append({
            "xT": np.ascontiguousarray(xT.reshape(IA, NBLK * IC * PB)),
            "w": Wf,
            "bb": bb,
        })
    res = bass_utils.run_bass_kernel_spmd(nc, in_maps,
                                          core_ids=list(range(NCORES)))
    outs = [res.results[c]["out"].reshape(BPC, H, Wd, NC, CD)
            for c in range(NCORES)]
    return np.concatenate(outs, axis=0)



# revision 74
# speedup vs baseline: 1.0434x; 1.0434x over previous
"""FCCapsuleLayer (dynamic routing, 3 iters) Trainium2 Bass kernel.

Sharding: data-parallel over batch, 8 cores x 4 batches. Per core 1024
positions (4*16*16). Routing is local per position, so no cross-core
communication is needed.

Per-core program (8 blocks of 128 positions, pos on SBUF partitions):
  - votes[pos, i, nc, cd] = x[pos,i,:] @ W    via PE: for each i, one
    matmul with stationary xT_i [a=16, pos=128] and moving W [16, 160].
    PSUM->SBUF evacuation on ScalarE (keeps DVE free).
  - 3 routing iterations on DVE with strided-view segmented reduces:
      iter1: route uniform -> preact = 0.1*sum_i votes + b
      agreement: tmp = votes * act  -> reduce over cd -> logits
      softmax over nc; preact = reduce_i (votes * route) + b; squash.
Host side: shard, pre-transpose x to [a, blk, i, pos]; gather outputs.
"""

from contextlib import ExitStack

import numpy as np

import concourse.bacc as bacc
import concourse.bass as bass
import concourse.tile as tile
from concourse import bass_utils, mybir

F32 = mybir.dt.float32
AX = mybir.AxisListType
OP = mybir.AluOpType

B, H, Wd, IC, IA = 32, 16, 16, 32, 16
NC, CD = 10, 16
NCD = NC * CD  # 160
NCORES = 8
BPC = B // NCORES          # batches per core
POS = BPC * H * Wd         # 1024 positions per core
PB = 128                   # positions per block
NBLK = POS // PB           # 8
EPS = 1e-7
IGRP = 3                   # i's per PSUM tile (3*160*4B = 1920B < 2KB bank)

_PROG_CACHE = {}


def _squash_emit(nc, pool, pre, tag, eps_s=None):
    """pre: [128, NCD] tile AP viewed [p, nc, cd]. Returns act tile [128, NCD]."""
    psq = pool.tile([PB, NCD], F32, tag=f"psq{tag}")
    nc.scalar.square(psq[:], pre[:])
    sq = pool.tile([PB, NC], F32, tag=f"sq{tag}")
    nc.vector.tensor_reduce(
        sq[:], psq[:].rearrange("p (n c) -> p n c", n=NC, c=CD),
        axis=AX.X, op=OP.add)
    t1 = pool.tile([PB, NC], F32, tag=f"t1{tag}")
    nc.vector.tensor_scalar_add(t1[:], sq[:], 1.0)
    r1 = pool.tile([PB, NC], F32, tag=f"r1{tag}")
    nc.vector.reciprocal(r1[:], t1[:])
    fa = pool.tile([PB, NC], F32, tag=f"fa{tag}")
    nc.vector.tensor_mul(fa[:], sq[:], r1[:])          # sq/(1+sq)
    lg = pool.tile([PB, NC], F32, tag=f"lg{tag}")
    nc.scalar.activation(lg[:], sq[:], mybir.ActivationFunctionType.Ln,
                         bias=eps_s[:])                # ln(sq+eps)
    r2 = pool.tile([PB, NC], F32, tag=f"r2{tag}")
    nc.scalar.activation(r2[:], lg[:], mybir.ActivationFunctionType.Exp,
                         scale=-0.5)                   # rsqrt(sq+eps)
    f = pool.tile([PB, NC], F32, tag=f"f{tag}")
    nc.vector.tensor_mul(f[:], fa[:], r2[:])
    act = pool.tile([PB, NCD], F32, tag=f"act{tag}")
    fb = f[:].unsqueeze(2).broadcast_to((PB, NC, CD))
    nc.vector.tensor_mul(
        act[:].rearrange("p (n c) -> p n c", n=NC, c=CD),
        pre[:].rearrange("p (n c) -> p n c", n=NC, c=CD), fb)
    return act


def _softmax_emit(nc, pool, logits, tag):
    """logits: [128, IC*NC] viewed [p, i, nc]. Returns route tile [128, IC*NC]."""
    e = pool.tile([PB, IC * NC], F32, tag=f"e{tag}")
    ev = e[:].rearrange("p (i n) -> p i n", i=IC, n=NC)
    nc.scalar.activation(e[:], logits[:], mybir.ActivationFunctionType.Exp)
    d = pool.tile([PB, IC], F32, tag=f"d{tag}")
    nc.vector.tensor_reduce(d[:], ev, axis=AX.X, op=OP.add)
    r = pool.tile([PB, IC], F32, tag=f"r{tag}")
    nc.vector.reciprocal(r[:], d[:])
    route = pool.tile([PB, IC * NC], F32, tag=f"route{tag}")
    rb = r[:].unsqueeze(2).broadcast_to((PB, IC, NC))
    nc.vector.tensor_mul(
        route[:].rearrange("p (i n) -> p i n", i=IC, n=NC), ev, rb)
    return route


def _build_program():
    nc = bacc.Bacc("TRN2", target_bir_lowering=False, debug=False,
                   enable_asserts=False, num_devices=NCORES)
    xT_d = nc.dram_tensor("xT", [IA, NBLK * IC * PB], F32,
                          kind="ExternalInput").ap()
    w_d = nc.dram_tensor("w", [IA, NCD], F32, kind="ExternalInput").ap()
    bb_d = nc.dram_tensor("bb", [PB, NCD], F32, kind="ExternalInput").ap()
    out_d = nc.dram_tensor("out", [POS, NCD], F32, kind="ExternalOutput").ap()

    with tile.TileContext(nc) as tc, ExitStack() as ctx:
        const = ctx.enter_context(tc.tile_pool(name="const", bufs=1))
        w_s = const.tile([IA, NCD], F32)
        nc.sync.dma_start(w_s[:], w_d)
        bb_s = const.tile([PB, NCD], F32)
        nc.sync.dma_start(bb_s[:], bb_d)
        eps_s = const.tile([PB, 1], F32)
        nc.vector.memset(eps_s[:], EPS)
        zero_s = const.tile([PB, 1], F32)
        nc.vector.memset(zero_s[:], 0.0)
        nc.const_aps.aps[(F32, 0.0)] = zero_s[:]

        xt_pool = ctx.enter_context(tc.tile_pool(name="xt", bufs=3))
        votes_pool = ctx.enter_context(tc.tile_pool(name="votes", bufs=2))
        tmp_pool = ctx.enter_context(tc.tile_pool(name="tmp", bufs=3))
        sm = ctx.enter_context(tc.tile_pool(name="small", bufs=4))
        psum = ctx.enter_context(tc.tile_pool(name="ps", bufs=6, space="PSUM"))
        spsum = ctx.enter_context(tc.tile_pool(name="sps", bufs=2, space="PSUM"))

        def emit_front(blk):
            """Votes (PE) + evacuation (ScalarE) + iter-1 squash (small DVE)
            + B1 agreement multiply (GPSIMD). Emitted one block ahead so
            these fill the other engines while DVE grinds block blk-1."""
            xt = xt_pool.tile([IA, IC * PB], F32)
            base = blk * IC * PB
            if blk <= 1:
                # chunk the fill-critical input DMAs so the PE can start on
                # the first quarter instead of waiting for the full 256KB
                q = IC * PB // 4
                for c4 in range(4):
                    nc.sync.dma_start(xt[:, c4 * q:(c4 + 1) * q],
                                      xT_d[:, base + c4 * q:base + (c4 + 1) * q])
            else:
                nc.sync.dma_start(xt[:], xT_d[:, base:base + IC * PB])
            votes = votes_pool.tile([PB, IC * NCD], F32)
            # Blocks 0/1 sit on the serial prologue: skip their duplicated
            # PE accumulation matmuls (halves the PE critical path there)
            # and sum votes over i on the DVE, which is idle during fill.
            sps = None if blk <= 1 else spsum.tile([PB, NCD], F32, tag="sps")
            i = 0
            while i < IC:
                ni = min(IGRP, IC - i)
                ps = psum.tile([PB, IGRP * NCD], F32, tag="vps")
                for k in range(ni):
                    nc.tensor.matmul(
                        ps[:, k * NCD:(k + 1) * NCD],
                        lhsT=xt[:, (i + k) * PB:(i + k + 1) * PB],
                        rhs=w_s[:], start=True, stop=True)
                    if sps is not None:
                        nc.tensor.matmul(
                            sps[:], lhsT=xt[:, (i + k) * PB:(i + k + 1) * PB],
                            rhs=w_s[:], start=(i + k == 0),
                            stop=(i + k == IC - 1), skip_group_check=True)
                nc.scalar.copy(votes[:, i * NCD:(i + ni) * NCD],
                               ps[:, :ni * NCD])
                i += ni

            v_inc = votes[:].rearrange("p (i n c) -> p i n c", i=IC, n=NC, c=CD)

            # iter 1: uniform route
            pre = sm.tile([PB, NCD], F32, tag="pre")
            if sps is None:
                # split the i-sum into quarters so each reduce can start as
                # soon as its slice of the evacuation has landed (subtile
                # deps); the extra adds run in prologue idle time
                qt = IC // 4
                parts = []
                for p4 in range(4):
                    hq = sm.tile([PB, NCD], F32, tag=f"presum{p4}")
                    nc.vector.tensor_reduce(
                        hq[:].rearrange("p (n c) -> p n c", n=NC, c=CD),
                        votes[:, p4 * qt * NCD:(p4 + 1) * qt * NCD].rearrange(
                            "p (i n c) -> p n c i", i=qt, n=NC, c=CD),
                        axis=AX.X, op=OP.add)
                    parts.append(hq)
                nc.vector.tensor_add(parts[0][:], parts[0][:], parts[1][:])
                nc.vector.tensor_add(parts[2][:], parts[2][:], parts[3][:])
                nc.vector.tensor_add(parts[0][:], parts[0][:], parts[2][:])
                nc.vector.scalar_tensor_tensor(
                    pre[:], parts[0][:], 1.0 / NC, bb_s[:],
                    op0=OP.mult, op1=OP.add)
            else:
                nc.vector.scalar_tensor_tensor(
                    pre[:], sps[:], 1.0 / NC, bb_s[:], op0=OP.mult, op1=OP.add)
            act = _squash_emit(nc, sm, pre, "a", eps_s)

            # agreement 1 multiply. Steady state: GPSIMD + cd-fold 16->4,
            # fully off the DVE chain (prefetched one block ahead). Block 0
            # has no prior block to overlap with, so the slow GPSIMD path
            # would sit on the critical prologue: do it on DVE instead.
            tmp = tmp_pool.tile([PB, IC * NCD], F32, tag="tmp")
            ab = act[:].rearrange("p (n c) -> p n c", n=NC, c=CD).unsqueeze(1) \
                .broadcast_to((PB, IC, NC, CD))
            eng = nc.vector if blk <= 1 else nc.gpsimd
            eng.tensor_mul(
                tmp[:].rearrange("p (i n c) -> p i n c", i=IC, n=NC, c=CD),
                v_inc, ab)
            if blk > 1:
                t4 = tmp[:].rearrange("p (i n f c) -> p i n f c", i=IC, n=NC,
                                      f=2, c=8)
                nc.gpsimd.tensor_add(t4[:, :, :, 0, :], t4[:, :, :, 0, :],
                                     t4[:, :, :, 1, :])
                t8 = tmp[:].rearrange("p (i n f c) -> p i n f c", i=IC, n=NC,
                                      f=4, c=4)
                nc.gpsimd.tensor_add(t8[:, :, :, 0, :], t8[:, :, :, 0, :],
                                     t8[:, :, :, 1, :])
            return votes, v_inc, tmp, blk

        def emit_back(blk, votes, v_inc, tmp, fb):
            logits = sm.tile([PB, IC * NC], F32, tag="logits")
            tq = tmp[:].rearrange("p (i n c) -> p i n c", i=IC, n=NC, c=CD)
            nfold = CD if fb <= 1 else 4
            nc.vector.tensor_reduce(
                logits[:].rearrange("p (i n) -> p i n", i=IC, n=NC),
                tq[:, :, :, 0:nfold], axis=AX.X, op=OP.add)

            for it in (2, 3):
                route = _softmax_emit(nc, sm, logits, f"it{it}")
                rb = route[:].rearrange("p (i n) -> p i n", i=IC, n=NC).unsqueeze(3) \
                    .broadcast_to((PB, IC, NC, CD))
                tmp2 = tmp_pool.tile([PB, IC * NCD], F32, tag="tmp")
                nc.vector.tensor_mul(
                    tmp2[:].rearrange("p (i n c) -> p i n c",
                                      i=IC, n=NC, c=CD),
                    v_inc, rb)
                pre2 = sm.tile([PB, NCD], F32, tag="pre")
                nc.vector.tensor_reduce(
                    pre2[:].rearrange("p (n c) -> p n c", n=NC, c=CD),
                    tmp2[:].rearrange("p (i n c) -> p n c i",
                                      i=IC, n=NC, c=CD),
                    axis=AX.X, op=OP.add)
                nc.vector.tensor_add(pre2[:], pre2[:], bb_s[:])
                act = _squash_emit(nc, sm, pre2, "a", eps_s)
                if it < 3:
                    tmp3 = tmp_pool.tile([PB, IC * NCD], F32, tag="tmp")
                    ab2 = act[:].rearrange("p (n c) -> p n c", n=NC, c=CD).unsqueeze(1) \
                        .broadcast_to((PB, IC, NC, CD))
                    nc.vector.tensor_mul(
                        tmp3[:].rearrange("p (i n c) -> p i n c",
                                          i=IC, n=NC, c=CD),
                        v_inc, ab2)
                    agree = sm.tile([PB, IC * NC], F32, tag="agree")
                    nc.vector.tensor_reduce(
                        agree[:].rearrange("p (i n) -> p i n", i=IC, n=NC),
                        tmp3[:].rearrange("p (i n c) -> p i n c",
                                          i=IC, n=NC, c=CD),
                        axis=AX.X, op=OP.add)
                    logits2 = sm.tile([PB, IC * NC], F32, tag="logits")
                    nc.vector.tensor_add(logits2[:], logits[:], agree[:])
                    logits = logits2

            nc.sync.dma_start(out_d[blk * PB:(blk + 1) * PB, :], act[:])

        state = {}
        for blk in range(NBLK + 1):
            if blk < NBLK:
                state[blk] = emit_front(blk)
            if blk >= 1:
                v, vi, t, fb = state.pop(blk - 1)
                emit_back(blk - 1, v, vi, t, fb)
    # Pin every ScalarE activation to the one table set that contains all
    # functions we use (exp, ln, square, copy, identity) so the act-table
    # insertion pass emits a single hoisted load instead of thrashing
    # between sets on every softmax/squash (~2.7us per reload).
    _orig_gat = bacc.get_activation_tables
    _ONE_SET = "natural_log_exp_and_others"

    def _pinned(arch):
        tabs = _orig_gat(arch)
        return {k: (v if k == _ONE_SET else set()) for k, v in tabs.items()}

    bacc.get_activation_tables = _pinned
    try:
        nc.compile()
    finally:
        bacc.get_activation_tables = _orig_gat
    return nc


def _get_program():
    if "nc" not in _PROG_CACHE:
        _PROG_CACHE["nc"] = _build_program()
    return _PROG_CACHE["nc"]


def kernel(input_tensor: np.ndarray, W: np.ndarray, b: np.ndarray,
           **_ignored) -> np.ndarray:
    nc = _get_program()
    x = np.asarray(input_tensor, np.float32)
    Wf = np.ascontiguousarray(np.asarray(W, np.float32))
    bb = np.ascontiguousarray(
        np.broadcast_to(np.asarray(b, np.float32).reshape(1, NCD), (PB, NCD)))

    in_maps = []
    for c in range(NCORES):
        xc = x[c * BPC:(c + 1) * BPC].reshape(POS, IC, IA)
        # [pos, i, a] -> [a, blk, i, pos]
        xT = xc.reshape(NBLK, PB, IC, IA).transpose(3, 0, 2, 1)
        in_maps.append({
            "xT": np.ascontiguousarray(xT.reshape(IA, NBLK * IC * PB)),
            "w": Wf,
            "bb": bb,
        })
    res = bass_utils.run_bass_kernel_spmd(nc, in_maps,
                                          core_ids=list(range(NCORES)))
    outs = [res.results[c]["out"].reshape(BPC, H, Wd, NC, CD)
            for c in range(NCORES)]
    return np.concatenate(outs, axis=0)

